# revision 30
# baseline (speedup 1.0000x reference)
"""MoE HyperNet linear layer on 8 Trainium2 NeuronCores.

Reference computation (B=4096, I=O=1024, C=128, E=8):
    h      = relu(cond @ g_w1 + g_b1)                # [B, 4E]
    gating = softmax(h @ g_w2 + g_b2, axis=1)        # [B, E]
    out    = einsum('be,beo->bo', gating,
                    einsum('bi,eio->beo', x, W)) + gating @ expert_biases

Strategy: data-parallel shard B across the 8 cores (512 rows each),
replicate weights. Per core, OUTPUT-side gating:

    y_e = x @ W_e            (PE, bf16 operands, fp32 PSUM accumulate)
    acc = y_e * g[:,e] + acc (DVE fused scalar_tensor_tensor; the gate is
                              a per-partition scalar since y_e is [b, o])

vs. the input-side xT*gate formulation this removes the gate broadcast
matmuls and the gating->matmul boot dependency: the first main matmul
only needs an x slice and a W slice from DMA.

PE work per core: 8 experts x 8 ic x 4 bc x 2 oh = 512 matmuls of
N=512 at 1 cycle/row (bf16) ~= 109 us @2.4GHz -- the roofline.
W is cast to bf16 on the host (layout+dtype prep is host-side, like
the transposes), halving HBM traffic to ~18 MB/core (~55 us at the
~350 GB/s per-core DMA rate), so DMA fully hides under the PE.

Boot: junk warm-up + dependency-staged filler matmuls keep the PE
clock ramped through every wait (the PE's small OoO lookahead hoists
dependency-free fillers past stalled instructions, so fillers are
gated on condT/hT). Gating runs in natural [b, e] orientation (g_b2
folded in via an appended ones row on hT; softmax along the free dim)
entirely before PSUM is claimed by the 8 per-(bc,oh) accumulator
banks (separate tiles: dep tracking is tile-granular, so a drain of
one bank must not block matmuls into another).

DMA: gpsimd's software-dynamic queue is ~3x the two hardware-DGE
queues but ramps slowly for the first ~MB, so expert 0 streams as 8
single-ic 256 KB chunks (matching consumption), interleaved with
x_ic0/x_ic1 at the queue head; x_ic2..7 pace in on the sync/scalar
hardware queues. Expert 0 runs ic-major (chunk pacing); experts 1-7
run bc/oh-major (W fully prefetched, drains land far from the next
expert's PSUM restart). Output halves store as they finalize; the
last batch-chunk's stores use the hardware queues so the gpsimd
queue's slow post-completion drain overlaps compute.

expert_biases are all-zero in the reference's setup_inputs; the host
checks and only emits the bias path (a K=1 ones-row matmul appended to
every accumulation chain, mathematically exact through the gate scale)
when some bias is nonzero.

Any instruction here can carry only ONE sync wait (walrus limit), so a
post-pass splits extra waits onto same-engine NoOps (_split_waits).
"""

import sys

if "/opt/trn_rl_repo" not in sys.path:
    sys.path.insert(0, "/opt/trn_rl_repo")

import ml_dtypes
import numpy as np

import bass_rust
import concourse.bass as bass
import concourse.mybir as mybir
import concourse.tile as tile
from concourse.bass_utils import run_bass_kernel_spmd

BF16 = ml_dtypes.bfloat16


def _split_waits(nc, max_waits=1):
    """Hoist all-but-one sync wait of each instruction onto same-engine
    predecessors. This walrus build rejects any TPB instruction carrying
    more than one wait ("Too many sync wait commands"); engines are
    in-order so the split preserves semantics.

    A matmul's own InstLdweight (emitted immediately before it, normally
    waitless) absorbs one spare wait for free — an inserted NoOp costs
    ~100-200 ns of PE sequencer issue time, which showed up as a 432 ns
    bubble at every expert boundary. Moving a wait one slot earlier on
    an in-order engine cannot deadlock unless the original program
    already did. Remaining spares still get NoOps."""
    for bb in nc.m.functions[0].blocks:
        out = []
        for i in list(bb.instructions):
            si = i.sync_info
            waits = list(si.on_wait) if si else []
            if len(waits) > max_waits:
                # find the same-engine immediate predecessor; absorb one
                # wait into it if it is this matmul's waitless Ldweight
                for j in range(len(out) - 1, -1, -1):
                    p = out[j]
                    if p.engine != i.engine:
                        continue
                    psi = p.sync_info
                    pw = list(psi.on_wait) if psi else []
                    if (type(p).__name__ == "InstLdweights"
                            and len(pw) < max_waits):
                        take = min(max_waits - len(pw),
                                   len(waits) - max_waits)
                        p.sync_info = bass_rust.SyncInfo(
                            on_wait=pw + waits[:take],
                            on_update=list(psi.on_update) if psi else [])
                        waits = waits[take:]
                    break
            if len(waits) > max_waits:
                for k, w in enumerate(waits[:-max_waits]):
                    nop = mybir.InstNoOp(
                        name=f"{i.name}-waitsplit{k}", ins=[], outs=[])
                    nop.engine = i.engine
                    nop.sync_info = bass_rust.SyncInfo(on_wait=[w], on_update=[])
                    out.append(nop)
            if si is not None and len(waits) != len(si.on_wait):
                i.sync_info = bass_rust.SyncInfo(
                    on_wait=waits[-max_waits:] if len(waits) > max_waits
                    else waits,
                    on_update=list(si.on_update))
            elif len(waits) > max_waits:
                i.sync_info = bass_rust.SyncInfo(
                    on_wait=waits[-max_waits:], on_update=list(si.on_update))
            out.append(i)
        bb.instructions = out


B, I, O, C, E = 4096, 1024, 1024, 128, 8
N_CORES = 8
BS = B // N_CORES          # 512 batch rows per core
NB = BS // 128             # 4 batch chunks of 128
NI = I // 128              # 8 contraction chunks
NQ = 4                     # W / xT quarter tiles (2 ic chunks each)
H = 4 * E                  # 32 gating hidden

_cache = {}


def _build_nc(has_bias):
    dt = mybir.dt
    f32, bf16 = dt.float32, dt.bfloat16

    nc = bass.Bass("TRN2", target_bir_lowering=False, debug=False,
                   num_devices=N_CORES)

    xT_d = nc.dram_tensor("xT_sh", [I, BS], bf16, kind="ExternalInput").ap()
    condT_d = nc.dram_tensor("condT_sh", [C, BS], bf16, kind="ExternalInput").ap()
    w_d = nc.dram_tensor("w", [E * I, O], bf16, kind="ExternalInput").ap()
    gpack_d = nc.dram_tensor("gpack", [128, 41], bf16, kind="ExternalInput").ap()
    gb1f_d = nc.dram_tensor("gb1f", [H, 1], f32, kind="ExternalInput").ap()
    if has_bias:
        eb_d = nc.dram_tensor("eb", [1, E * O], bf16, kind="ExternalInput").ap()
    out_d = nc.dram_tensor("out_sh", [BS, O], f32, kind="ExternalOutput").ap()

    with tile.TileContext(nc) as tc:
        with (
            tc.tile_pool(name="consts", bufs=1) as consts,
            tc.tile_pool(name="stage", bufs=1) as stage,
            tc.tile_pool(name="wpool", bufs=8) as wpool,
        ):
            junk = consts.tile([128, 256], bf16, tag="junk")
            zjunk = consts.tile([128, 128], bf16, tag="zjunk")
            gpack = consts.tile([128, 41], bf16, tag="gpack")
            gw1 = gpack[:, 0:H]            # [128, 32]
            gb1 = gpack[0:H, H:H + 1]      # [32, 1]
            gw2a = gpack[0:H + 1, 33:41]   # [33, 8] (last row = g_b2)
            condT = stage.tile([C, BS], bf16, tag="condT")
            gb1f = stage.tile([H, 1], f32, tag="gb1f")
            hT = stage.tile([H + 1, BS], bf16, tag="hT")
            ez = stage.tile([128, NB * E], f32, tag="ez")
            rden = stage.tile([128, NB], f32, tag="rden")
            rdenr = stage.tile([128, NB], f32, tag="rdenr")
            gcols = stage.tile([128, NB * E], f32, tag="gcols")
            if has_bias:
                ones1 = consts.tile([1, 128], bf16, tag="ones1")
                ebt = stage.tile([1, E * O], bf16, tag="ebt")
            # x: 8 single-ic tiles (256 KB each; dep tracking is
            # tile-granular) alternating across the two hardware DMA
            # queues, so each slice lands just before the PE needs it.
            # xmap[ic] -> (tile, j): lhsT = tile[:, j*BS + bc*128 ...]
            xtiles = [stage.tile([128, BS], bf16, tag=f"x{ic}",
                                 name=f"x{ic}") for ic in range(NI)]
            xmap = [(xtiles[ic], 0) for ic in range(NI)]
            accs = [stage.tile([128, O], f32, tag=f"acc{bc}",
                               name=f"acc{bc}") for bc in range(NB)]

            # W chunk issue helper; expert 0's first two chunks are
            # single-ic tiles (256 KB) so the first matmul starts ASAP.
            # wmap[ic] -> (tile, j): rhs = tile[:, j*O + oh*512 ..]
            def issue_w_chunk(e, ci, ic0, n):
                wt = wpool.tile([128, n * O], bf16, tag=f"w{n}",
                                name=f"w{e}c{ci}", bufs=8)
                rows = w_d[e * I + ic0 * 128:e * I + (ic0 + n) * 128, :]
                nc.gpsimd.dma_start(
                    wt[:].rearrange("p (j o) -> p j o", j=n),
                    rows.rearrange("(j p) o -> p j o", p=128))
                return [(wt, j) for j in range(n)]

            # ---- DMAs, priority order per queue ----
            # gpsimd's software-dynamic queue is the fast one (~3x the
            # hardware queues): boot-critical x0/W0/x1/W1 interleaved
            # there in consumption order, then the W stream; x2-x7 on
            # the sync/scalar hardware queues, paced to consumption;
            # output stores reuse gpsimd at the tail.
            nc.vector.memset(junk[:], 1.0)  # warm-up dep, first on DVE
            nc.vector.memset(zjunk[:], 0.0)
            nc.vector.memset(hT[H:H + 1, :], 1.0)  # ones row for g_b2
            if has_bias:
                nc.vector.memset(ones1[:], 1.0)
                nc.scalar.dma_start(ebt[:], eb_d)
            nc.sync.dma_start(gpack[:], gpack_d)
            nc.sync.dma_start(gb1f[:], gb1f_d)
            nc.scalar.dma_start(condT[:], condT_d)
            xs3 = xT_d.rearrange("(ic p) b -> p ic b", p=128)
            # expert 0 streams as 8 single-ic W chunks: the early DMA
            # rate (~150 GB/s while the queue ramps) matches the PE's
            # 256 KB / 1.7 us consumption only at this granularity
            nc.gpsimd.dma_start(xtiles[0][:], xs3[:, 0, :])
            wmap0 = issue_w_chunk(0, 0, 0, 1)
            nc.gpsimd.dma_start(xtiles[1][:], xs3[:, 1, :])
            for ic in range(1, NI):
                wmap0 += issue_w_chunk(0, ic, ic, 1)
            for ic in range(2, NI):
                eng = nc.sync if ic % 2 == 0 else nc.scalar
                eng.dma_start(xtiles[ic][:], xs3[:, ic, :])

            with tc.tile_pool(name="ps_boot", bufs=1, space="PSUM") as ps_b:
                # HAM warm-up: keep the PE busy from engine boot so the
                # clock is ramped when the real matmuls arrive. No input
                # deps beyond the memset.
                pj = ps_b.tile([128, 256], f32, tag="pj")
                for i in range(14):
                    nc.tensor.matmul(pj[:], junk[:, 0:128], junk[:],
                                     start=(i == 0), stop=(i == 13))

                # ---- gating, natural [b, e] orientation ----
                ph = ps_b.tile([H, BS], f32, tag="ph")
                nc.tensor.matmul(ph[:], gw1, condT[:], start=True, stop=True)
                # fillers gated on condT: cover the relu window (the PE's
                # 4-deep OoO lookahead would hoist dependency-free ones
                # all the way past the gating chain, re-opening the gap)
                for i in range(4):
                    nc.tensor.matmul(pj[:], condT[0:128, 0:128], junk[:],
                                     start=(i == 0), stop=(i == 3))
                # hT[0:32] = relu(ph + g_b1) on DVE (the scalar engine is
                # busy issuing DMAs right then); row 32 stays 1.0
                nc.vector.tensor_scalar(hT[0:H, :], ph[:], gb1f[:], 0.0,
                                        mybir.AluOpType.add,
                                        mybir.AluOpType.max)
                pg = ps_b.tile([128, NB * E], f32, tag="pg")
                for bc in range(NB):
                    nc.tensor.matmul(pg[:, bc * E:(bc + 1) * E],
                                     hT[:, bc * 128:(bc + 1) * 128], gw2a,
                                     start=True, stop=True)
                # fillers gated on hT: bridge softmax -> first-W-arrival;
                # slightly over-provisioned so the main stream starts with
                # ~0.3 MB of x/W buffered (absorbs early DMA-ramp jitter
                # that would otherwise stall the PE and drop its p-state)
                for i in range(11):
                    nc.tensor.matmul(pj[:], hT[0:H + 1, 0:128],
                                     junk[0:H + 1, :],
                                     start=(i == 0), stop=(i == 10))
                # absorb the x0/W0c0 first-reader wait edges on throwaway
                # matmuls (the PE waits for that data here regardless), so
                # the first real matmul issues with no co-located edge
                # pair (a 2-edge instruction stalls issue ~212 ns)
                nc.tensor.matmul(pj[:], xtiles[0][:, 0:128], junk[:],
                                 start=True, stop=False)
                nc.tensor.matmul(pj[:], junk[:, 0:128],
                                 wmap0[0][0][:, 0:256],
                                 start=False, stop=True)
                nc.scalar.activation(ez[:], pg[:],
                                     mybir.ActivationFunctionType.Exp,
                                     bias=0.0, scale=1.0)
                nc.vector.tensor_reduce(
                    rden[:], ez[:].rearrange("p (n e) -> p n e", e=E),
                    mybir.AxisListType.X, mybir.AluOpType.add)
                nc.vector.reciprocal(rdenr[:], rden[:])
                for bc in range(NB):
                    nc.vector.tensor_scalar(
                        gcols[:, bc * E:(bc + 1) * E],
                        ez[:, bc * E:(bc + 1) * E],
                        rdenr[:, bc:bc + 1], 0.0,
                        mybir.AluOpType.mult, mybir.AluOpType.add)

            # ---- main loop: per-expert GEMMs + gated drains ----
            with tc.tile_pool(name="ps_main", bufs=1, space="PSUM") as ps_main:
                # one PSUM tile per (bc, oh) bank: dep tracking is
                # tile-granular, so a drain reading one O-half must not
                # block the next expert's matmuls into the other half
                pouts = [[ps_main.tile([128, 512], f32, tag=f"po{bc}_{oh}",
                                       name=f"po{bc}_{oh}")
                          for oh in range(2)] for bc in range(NB)]
                def mm(e, ic, bc, oh, wmap, start, stop):
                    xt, xj = xmap[ic]
                    wt, wj = wmap[ic]
                    nc.tensor.matmul(
                        pouts[bc][oh][:],
                        xt[:, xj * BS + bc * 128:xj * BS + (bc + 1) * 128],
                        wt[:, wj * O + oh * 512:wj * O + (oh + 1) * 512],
                        start=start, stop=stop)

                def bias_mm(e, bc, oh):
                    # exact through the gate scale: y_e += eb_e
                    nc.tensor.matmul(
                        pouts[bc][oh][:],
                        ones1[0:1, 0:128],
                        ebt[0:1, e * O + oh * 512:e * O + (oh + 1) * 512],
                        start=False, stop=True)

                def drain(e, bc, oh, k0=0, k1=512):
                    g = gcols[:, bc * E + e:bc * E + e + 1]
                    sl = slice(oh * 512 + k0, oh * 512 + k1)
                    if e == 0:
                        nc.vector.tensor_scalar(
                            accs[bc][:, sl], pouts[bc][oh][:, k0:k1], g, 0.0,
                            mybir.AluOpType.mult, mybir.AluOpType.add)
                    else:
                        nc.vector.scalar_tensor_tensor(
                            accs[bc][:, sl], pouts[bc][oh][:, k0:k1], g,
                            accs[bc][:, sl],
                            mybir.AluOpType.mult, mybir.AluOpType.add)

                for e in range(E):
                    if e == 0:
                        wmap = wmap0  # issued up-front, interleaved with x
                    else:
                        wmap = []
                        for q in range(NQ):
                            wmap += issue_w_chunk(e, q, 2 * q, 2)
                    if e == 0:
                        # ic-major: each W chunk feeds 8-16 matmuls while
                        # the next one streams in
                        for ic in range(NI):
                            for bc in range(NB):
                                for oh in range(2):
                                    mm(e, ic, bc, oh, wmap,
                                       start=(ic == 0),
                                       stop=(ic == NI - 1 and not has_bias))
                                if ic == NI - 1:
                                    for oh in range(2):
                                        if has_bias:
                                            bias_mm(e, bc, oh)
                                        drain(e, bc, oh)
                    else:
                        # W is prefetched well ahead by now: bc-major and
                        # oh-major inside, so each half's drain (+ store
                        # for the last expert) overlaps the next half's
                        # matmuls and sits far from the next expert's
                        # restart of the same PSUM bank
                        for bc in range(NB):
                            for oh in range(2):
                                for ic in range(NI):
                                    mm(e, ic, bc, oh, wmap,
                                       start=(ic == 0),
                                       stop=(ic == NI - 1 and not has_bias))
                                if has_bias:
                                    bias_mm(e, bc, oh)
                                if e < E - 1:
                                    drain(e, bc, oh)
                                elif bc < NB - 1:
                                    drain(e, bc, oh)
                                    sl = slice(oh * 512, (oh + 1) * 512)
                                    nc.gpsimd.dma_start(
                                        out_d[bc * 128:(bc + 1) * 128, sl],
                                        accs[bc][:, sl])
                                else:
                                    # last bc's stores go on the idle
                                    # sync/scalar hardware queues (which
                                    # complete crisply) so the gpsimd
                                    # queue's slow post-completion drain
                                    # happens during compute, not after.
                                    # The final half goes in two pieces so
                                    # the second drain overlaps the first
                                    # store's descriptor generation.
                                    pieces = ([(0, 512, nc.sync)] if oh == 0
                                              else [(0, 256, nc.scalar),
                                                    (256, 512, nc.sync)])
                                    for k0, k1, eng in pieces:
                                        drain(e, bc, oh, k0, k1)
                                        sl = slice(oh * 512 + k0,
                                                   oh * 512 + k1)
                                        eng.dma_start(
                                            out_d[bc * 128:(bc + 1) * 128, sl],
                                            accs[bc][:, sl])

    _split_waits(nc)
    return nc


def _get_nc(has_bias):
    key = ("nc", has_bias)
    if key not in _cache:
        _cache[key] = _build_nc(has_bias)
    return _cache[key]


def _make_in_maps(x, cond, expert_weights, expert_biases, g_w1, g_b1, g_w2, g_b2,
                  has_bias):
    w_flat = np.ascontiguousarray(
        np.asarray(expert_weights, dtype=np.float32).reshape(E * I, O)
        .astype(BF16))
    xT = np.asarray(x, dtype=np.float32).T.astype(BF16)        # [I, B]
    condT = np.asarray(cond, dtype=np.float32).T.astype(BF16)  # [C, B]
    gpack = np.zeros((128, 41), dtype=np.float32)
    gpack[:, 0:H] = np.asarray(g_w1, dtype=np.float32)
    gpack[0:H, H] = np.asarray(g_b1, dtype=np.float32)
    gpack[0:H, 33:41] = np.asarray(g_w2, dtype=np.float32)
    gpack[H, 33:41] = np.asarray(g_b2, dtype=np.float32)
    common = {"w": w_flat, "gpack": gpack.astype(BF16),
              "gb1f": np.asarray(g_b1, dtype=np.float32).reshape(H, 1)}
    if has_bias:
        common["eb"] = np.ascontiguousarray(
            np.asarray(expert_biases, dtype=np.float32).astype(BF16)
            .reshape(1, E * O))
    in_maps = []
    for c in range(N_CORES):
        m = dict(common)
        m["xT_sh"] = np.ascontiguousarray(xT[:, c * BS:(c + 1) * BS])
        m["condT_sh"] = np.ascontiguousarray(condT[:, c * BS:(c + 1) * BS])
        in_maps.append(m)
    return in_maps


def run(inputs, trace=False, **kw):
    """Build + run; returns (full_out [B, O] fp32, BassKernelResults)."""
    has_bias = bool(np.any(np.asarray(inputs["expert_biases"])))
    nc = _get_nc(has_bias)
    in_maps = _make_in_maps(**inputs, has_bias=has_bias)
    res = run_bass_kernel_spmd(nc, in_maps, core_ids=list(range(N_CORES)),
                               trace=trace, **kw)
    out = np.concatenate([res.results[c]["out_sh"] for c in range(N_CORES)],
                         axis=0)
    return out, res


def kernel(**inputs):
    out, _ = run(inputs)
    return out
